# revision 32
# baseline (speedup 1.0000x reference)
"""Multi-head masked attention on 8 Trainium2 NeuronCores.

Sharding: data-parallel over batch (B=2 -> 2 groups of 4 cores),
tensor-parallel over heads within a group (16 heads -> 4 heads/core).
Each core computes q/k/v projections for its 4 heads (column-sharded),
causal flash-style attention in the transposed (S^T) domain, and a
row-sharded partial o-projection. The host sums the 4 partials per
batch element and adds the output bias.

v3: bf16 matmul operands (fp32 PSUM accumulate), x^T via the DMA XBAR
transpose, causal-trimmed QK/exp, mask as post-exp 0/1 multiply on
gpsimd, and a software-pipelined emission schedule: PV runs two
iterations behind QK, and projection / o-projection / x-pipeline work
for other windows is drained as PE filler into the exp-wait slots so
the tensor engine stays dense. Host pre-scales wq/bq by 1/sqrt(Dh).

Self-contained: hardcodes shapes B=2, T=2048, C=1024, H=16, Dh=64.
"""

import sys

sys.path.insert(0, "/opt/trn_rl_repo")

import numpy as np

import concourse.bass as bass
import concourse.tile as tile
import concourse.mybir as mybir
from concourse import bacc
from concourse.bass import ts, ds

F32 = mybir.dt.float32
BF16 = mybir.dt.bfloat16
AF = mybir.ActivationFunctionType
ALU = mybir.AluOpType

B, T, C = 2, 2048, 1024
H, DH = 16, 64
HPC = 4            # heads per core
DQC = HPC * DH     # 256 projected dims per core
N_CORES = 8

TC = T // 128      # 16 t-chunks of 128
CC = C // 128      # 8 c-chunks
TJ = T // 512      # 4 t-windows of 512


def build_program():
    nc = bacc.Bacc("TRN2", target_bir_lowering=False, debug=False)

    xb16 = nc.dram_tensor("xb16", [T, C], BF16, kind="ExternalInput")
    wq = nc.dram_tensor("wq", [C, DQC], F32, kind="ExternalInput")
    wk = nc.dram_tensor("wk", [C, DQC], F32, kind="ExternalInput")
    wv = nc.dram_tensor("wv", [C, DQC], F32, kind="ExternalInput")
    wo = nc.dram_tensor("wo", [DQC, C], F32, kind="ExternalInput")
    bq = nc.dram_tensor("bq", [DQC], F32, kind="ExternalInput")
    bk = nc.dram_tensor("bk", [DQC], F32, kind="ExternalInput")
    bv = nc.dram_tensor("bv", [DQC], F32, kind="ExternalInput")
    out = nc.dram_tensor("out", [T, C], F32, kind="ExternalOutput")

    with tile.TileContext(nc) as tc:
        with (
            tc.tile_pool(name="persist", bufs=1) as pp,
            tc.tile_pool(name="ps_s", bufs=2, space="PSUM") as ps_s,
            tc.tile_pool(name="ps_pv", bufs=2, space="PSUM") as ps_pv,
            tc.tile_pool(name="ps_misc", bufs=2, space="PSUM") as ps_misc,
            tc.tile_pool(name="psb", bufs=4) as pexp,
            tc.tile_pool(name="small", bufs=2) as psm,
            tc.tile_pool(name="outp", bufs=3) as pout,
        ):
            # ---- persistent sbuf tensors -------------------------------
            # xT[p, h, cc, t_local] = x[h*256 + t_local, cc*128 + p]
            xT = pp.tile([128, 2 * TJ, CC, 256], BF16, tag="xT")
            qT = pp.tile([128, 2, T], BF16, tag="qT")   # [p, half, t]
            kT = pp.tile([128, 2, T], BF16, tag="kT")
            vA = pp.tile([128, TC, HPC * (DH + 1)], BF16, tag="vA")
            yT = pp.tile([128, 2, T], BF16, tag="yT")
            wo_sb = pp.tile([128, 2, C], BF16, tag="wo")
            wq_sb = pp.tile([128, CC, DQC], BF16, tag="wq")
            wk_sb = pp.tile([128, CC, DQC], BF16, tag="wk")
            wv_sb = pp.tile([128, CC, DQC], BF16, tag="wv")
            wqf = pp.tile([128, CC, DQC], F32, tag="wqf")
            wkf = pp.tile([128, CC, DQC], F32, tag="wkf")
            wvf = pp.tile([128, CC, DQC], F32, tag="wvf")
            wof = pp.tile([128, 2, C], F32, tag="wof")
            bqs = pp.tile([128, 2], F32, tag="bqs")
            bks = pp.tile([128, 2], F32, tag="bks")
            bvs = pp.tile([128, DQC], F32, tag="bvs")

            # x^T first: XBAR transposes straight from DRAM on the sync
            # queue (everything downstream depends on these)
            for h in range(2 * TJ):
                nc.sync.dma_start_transpose(
                    xT[:, h], xb16.ap()[ts(h, 256), :]
                )

            # tri01[p, f] = 1 where f >= p else 0 (keep s<=t in diag blk)
            tri01 = pp.tile([128, 128], BF16, tag="tri01")
            nc.gpsimd.memset(tri01[:], 1.0)
            nc.gpsimd.affine_select(
                out=tri01[:],
                in_=tri01[:],
                compare_op=ALU.is_ge,
                fill=0.0,
                base=0,
                # keep where (-1)*p + f >= 0, i.e. f >= p
                pattern=[[1, 128]],
                channel_multiplier=-1,
            )

            # ones column of v_aug (softmax denominator via PV matmul)
            vA4 = vA[:].rearrange("p s (h d) -> p s h d", d=DH + 1)
            onesf = pp.tile([128, TC * HPC], F32, tag="onesf")
            nc.gpsimd.memset(onesf[:], 1.0)
            nc.vector.tensor_copy(
                vA4[:, :, :, DH : DH + 1],
                onesf[:].rearrange("p (s h o) -> p s h o", h=HPC, o=1),
            )

            # biases + weights on the scalar-engine DMA queue (parallel
            # with the x transposes on the sync queue)
            nc.scalar.dma_start(bqs[:], bq.ap().rearrange("(k p) -> p k", p=128))
            nc.scalar.dma_start(bks[:], bk.ap().rearrange("(k p) -> p k", p=128))
            nc.scalar.dma_start(
                bvs[0:1, :], bv.ap().rearrange("(o n) -> o n", o=1)
            )
            nc.gpsimd.partition_broadcast(bvs[:], bvs[0:1, :])

            nc.scalar.dma_start(
                wqf[:], wq.ap().rearrange("(c p) d -> p c d", p=128)
            )
            nc.scalar.dma_start(
                wkf[:], wk.ap().rearrange("(c p) d -> p c d", p=128)
            )
            nc.scalar.dma_start(
                wvf[:], wv.ap().rearrange("(c p) d -> p c d", p=128)
            )
            nc.scalar.dma_start(
                wof[:], wo.ap().rearrange("(k p) n -> p k n", p=128)
            )
            nc.vector.tensor_copy(wq_sb[:], wqf[:])
            nc.vector.tensor_copy(wk_sb[:], wkf[:])
            nc.vector.tensor_copy(wv_sb[:], wvf[:])
            nc.vector.tensor_copy(wo_sb[:], wof[:])

            # ---- filler generators (PE work to hide exp latency) -------
            def gen_qk_proj(w, hp, wsb, bias, dst):
                pq = ps_misc.tile([128, 512], F32, tag="misc", name=f"pj{w}{hp}")
                for cc in range(CC):
                    nc.tensor.matmul(
                        pq[:],
                        wsb[:, cc, ts(hp, 128)],
                        xT[:, 2 * w : 2 * w + 2, cc, :],
                        start=(cc == 0),
                        stop=(cc == CC - 1),
                    )
                    yield
                nc.vector.tensor_scalar(
                    dst[:, hp, ts(w, 512)],
                    pq[:],
                    bias[:, hp : hp + 1],
                    None,
                    ALU.add,
                )

            def gen_v_proj(sc):
                pv = ps_misc.tile([128, 512], F32, tag="misc", name=f"pv{sc}")
                for cc in range(CC):
                    nc.tensor.matmul(
                        pv[:, :DQC],
                        xT[:, sc // 2, cc, ds(128 * (sc % 2), 128)],
                        wv_sb[:, cc, :],
                        start=(cc == 0),
                        stop=(cc == CC - 1),
                    )
                    yield
                nc.vector.tensor_tensor(
                    vA4[:, sc, :, :DH],
                    pv[:, :DQC].rearrange("p (h d) -> p h d", d=DH),
                    bvs[:].rearrange("p (h d) -> p h d", d=DH),
                    ALU.add,
                )

            def gen_o_proj(w, tt):
                t0 = 512 * w + 128 * tt
                ot = pout.tile([128, C], F32, tag="o", name=f"o{w}{tt}")
                for nb in range(2):
                    po = ps_misc.tile(
                        [128, 512], F32, tag="misc", name=f"po{w}{tt}{nb}"
                    )
                    for kk in range(2):
                        nc.tensor.matmul(
                            po[:],
                            yT[:, kk, ds(t0, 128)],
                            wo_sb[:, kk, ts(nb, 512)],
                            start=(kk == 0),
                            stop=(kk == 1),
                        )
                        yield
                    nc.vector.tensor_copy(ot[:, ts(nb, 512)], po[:])
                    yield
                nc.sync.dma_start(out.ap()[ds(t0, 128), :], ot[:])

            def drain(q, n):
                steps = 0
                while q and steps < n:
                    try:
                        next(q[0])
                    except StopIteration:
                        q.pop(0)
                        continue
                    steps += 1

            # projections for window 0 (emitted directly)
            for g in (
                [gen_qk_proj(0, hp, wq_sb, bqs, qT) for hp in range(2)]
                + [gen_qk_proj(0, hp, wk_sb, bks, kT) for hp in range(2)]
                + [gen_v_proj(sc) for sc in range(4)]
            ):
                for _ in g:
                    pass

            # ---- window loop: attend w, filling with w+1 proj etc ------
            for w in range(TJ):
                fillers = []
                if w + 1 < TJ:
                    for hp in range(2):
                        fillers.append(gen_qk_proj(w + 1, hp, wq_sb, bqs, qT))
                        fillers.append(gen_qk_proj(w + 1, hp, wk_sb, bks, kT))
                    for sc in range(4 * (w + 1), 4 * (w + 1) + 4):
                        fillers.append(gen_v_proj(sc))
                # o-projections all deferred to att(3), the filler-starved
                # window (att(0..2) are fed by next-window projections)
                o_wins = {3: [0, 1, 2]}.get(w, [])
                for ow in o_wins:
                    for tt in range(4):
                        fillers.append(gen_o_proj(ow, tt))

                n_sc = 4 * (w + 1)
                total_steps = (64 if w + 1 < TJ else 0) + 24 * len(o_wins)
                per_iter = max(1, total_steps // (2 * n_sc))

                for hp in range(2):
                    hA, hB = 2 * hp, 2 * hp + 1
                    ppv_A = ps_pv.tile([128, 512], F32, tag="pv", name=f"pA{w}{hp}")
                    ppv_B = ps_pv.tile([128, 512], F32, tag="pv", name=f"pB{w}{hp}")

                    def emit_pv(sc, psb, off):
                        for hi, h in ((0, hA), (1, hB)):
                            ppv = ppv_A if hi == 0 else ppv_B
                            nc.tensor.matmul(
                                ppv[: DH + 1, ds(off, 512 - off)],
                                vA[:, sc, ds(h * (DH + 1), DH + 1)],
                                psb[:, ds(512 * hi + off, 512 - off)],
                                start=(sc == 0),
                                stop=(sc == n_sc - 1),
                            )

                    pending = []
                    for sc in range(n_sc):
                        k = sc - 4 * w  # >=0 on the causal diagonal
                        off = 128 * k if k > 0 else 0
                        pss = ps_s.tile([128, 1024], F32, tag="s", name=f"s{w}{hp}{sc}")
                        for hi in range(2):
                            half = 512 * hi
                            prow = slice(64 * hi, 64 * hi + 64)
                            nc.tensor.matmul(
                                pss[:, ds(half + off, 512 - off)],
                                kT[prow, hp, ts(sc, 128)],
                                qT[prow, hp, ds(512 * w + off, 512 - off)],
                                start=True,
                                stop=True,
                                tile_position=(64 * hi, 0),
                            )
                        psb = pexp.tile([128, 1024], BF16, tag="p", name=f"e{w}{hp}{sc}")
                        if off == 0:
                            nc.scalar.activation(psb[:], pss[:], AF.Exp)
                        else:
                            nc.scalar.activation(
                                psb[:, ds(off, 512 - off)],
                                pss[:, ds(off, 512 - off)],
                                AF.Exp,
                            )
                            nc.scalar.activation(
                                psb[:, ds(512 + off, 512 - off)],
                                pss[:, ds(512 + off, 512 - off)],
                                AF.Exp,
                            )
                        if k >= 0:
                            # zero s>t inside the diagonal blocks
                            for hi in range(2):
                                blk = ds(512 * hi + off, 128)
                                nc.gpsimd.tensor_tensor(
                                    psb[:, blk], psb[:, blk], tri01[:], ALU.mult
                                )
                        pending.append((sc, psb, off))
                        if len(pending) > 2:
                            emit_pv(*pending.pop(0))
                        # reserve the last iterations' quota for the tail
                        if sc < n_sc - 2:
                            drain(fillers, per_iter)
                    for args in pending:
                        emit_pv(*args)
                        drain(fillers, 4)

                    # normalize: y^T = Y_unnorm^T * (1/denom)
                    den = psm.tile([1, 1024], F32, tag="den")
                    for hi, ppv in ((0, ppv_A), (1, ppv_B)):
                        nc.vector.tensor_copy(
                            den[:, ts(hi, 512)], ppv[DH : DH + 1, :]
                        )
                    rec = psm.tile([1, 1024], F32, tag="rec")
                    scr = psm.tile([1, 1024], F32, tag="scr")
                    nc.vector.reciprocal_approx_accurate(rec[:], den[:], scr[:])
                    recB = psm.tile([DH, 1024], F32, tag="recB")
                    nc.gpsimd.partition_broadcast(recB[:], rec[:])
                    for hi, h in ((0, hA), (1, hB)):
                        ppv = ppv_A if hi == 0 else ppv_B
                        nc.vector.tensor_tensor(
                            yT[ds(64 * (h % 2), DH), h // 2, ts(w, 512)],
                            ppv[:DH, :],
                            recB[:, ts(hi, 512)],
                            ALU.mult,
                        )
                    drain(fillers, 6)
                drain(fillers, 1 << 30)

            # last window's o-projection
            for tt in range(4):
                for _ in gen_o_proj(TJ - 1, tt):
                    pass

    nc.compile()
    return nc


_CACHE = {}


def _get_program():
    if "nc" not in _CACHE:
        _CACHE["nc"] = build_program()
    return _CACHE["nc"]


def make_in_maps(x, wq, bq, wk, bk, wv, bv, wo):
    import ml_dtypes

    scale = np.float32(1.0 / np.sqrt(DH))
    x16 = np.ascontiguousarray(x).astype(ml_dtypes.bfloat16)
    in_maps = []
    for core in range(N_CORES):
        b, g = core // 4, core % 4
        sl = slice(g * DQC, (g + 1) * DQC)
        in_maps.append(
            {
                "xb16": x16[b],
                "wq": np.ascontiguousarray(wq[:, sl]) * scale,
                "wk": np.ascontiguousarray(wk[:, sl]),
                "wv": np.ascontiguousarray(wv[:, sl]),
                "wo": np.ascontiguousarray(wo[sl, :]),
                "bq": np.ascontiguousarray(bq[sl]) * scale,
                "bk": np.ascontiguousarray(bk[sl]),
                "bv": np.ascontiguousarray(bv[sl]),
            }
        )
    return in_maps


def kernel(x, wq, bq, wk, bk, wv, bv, wo, bo):
    from concourse import bass_utils

    x = np.asarray(x, dtype=np.float32)
    wq = np.asarray(wq, dtype=np.float32)
    wk = np.asarray(wk, dtype=np.float32)
    wv = np.asarray(wv, dtype=np.float32)
    wo = np.asarray(wo, dtype=np.float32)
    bq = np.asarray(bq, dtype=np.float32)
    bk = np.asarray(bk, dtype=np.float32)
    bv = np.asarray(bv, dtype=np.float32)
    bo = np.asarray(bo, dtype=np.float32)

    nc = _get_program()
    in_maps = make_in_maps(x, wq, bq, wk, bk, wv, bv, wo)
    res = bass_utils.run_bass_kernel_spmd(
        nc, in_maps, core_ids=list(range(N_CORES))
    )
    y = np.zeros((B, T, C), dtype=np.float32)
    for core in range(N_CORES):
        y[core // 4] += res.results[core]["out"]
    y += bo
    return y


# revision 41
# speedup vs baseline: 1.0705x; 1.0705x over previous
"""Multi-head masked attention on 8 Trainium2 NeuronCores.

Sharding: data-parallel over batch (B=2 -> 2 groups of 4 cores),
tensor-parallel over heads within a group (16 heads -> 4 heads/core).
Each core computes q/k/v projections for its 4 heads (column-sharded),
causal flash-style attention in the transposed (S^T) domain, and a
row-sharded partial o-projection. The host sums the 4 partials per
batch element and adds the output bias.

v3: bf16 matmul operands (fp32 PSUM accumulate), x^T via the DMA XBAR
transpose, causal-trimmed QK/exp, mask as post-exp 0/1 multiply on
gpsimd, and a software-pipelined emission schedule: PV runs two
iterations behind QK, and projection / o-projection / x-pipeline work
for other windows is drained as PE filler into the exp-wait slots so
the tensor engine stays dense. Host pre-scales wq/bq by 1/sqrt(Dh).

Self-contained: hardcodes shapes B=2, T=2048, C=1024, H=16, Dh=64.
"""

import sys

sys.path.insert(0, "/opt/trn_rl_repo")

import numpy as np

import concourse.bass as bass
import concourse.tile as tile
import concourse.mybir as mybir
from concourse import bacc
from concourse.bass import ts, ds

F32 = mybir.dt.float32
BF16 = mybir.dt.bfloat16
AF = mybir.ActivationFunctionType
ALU = mybir.AluOpType

B, T, C = 2, 2048, 1024
H, DH = 16, 64
HPC = 4            # heads per core
DQC = HPC * DH     # 256 projected dims per core
N_CORES = 8

TC = T // 128      # 16 t-chunks of 128
CC = C // 128      # 8 c-chunks
TJ = T // 512      # 4 t-windows of 512


def build_program():
    nc = bacc.Bacc("TRN2", target_bir_lowering=False, debug=False)

    xt16 = nc.dram_tensor("xt16", [C, T], BF16, kind="ExternalInput")
    wq = nc.dram_tensor("wq", [C, DQC], F32, kind="ExternalInput")
    wk = nc.dram_tensor("wk", [C, DQC], F32, kind="ExternalInput")
    wv = nc.dram_tensor("wv", [C, DQC], F32, kind="ExternalInput")
    wo = nc.dram_tensor("wo", [DQC, C], F32, kind="ExternalInput")
    bq = nc.dram_tensor("bq", [DQC], F32, kind="ExternalInput")
    bk = nc.dram_tensor("bk", [DQC], F32, kind="ExternalInput")
    bv = nc.dram_tensor("bv", [DQC], F32, kind="ExternalInput")
    out = nc.dram_tensor("out", [T, C], F32, kind="ExternalOutput")

    with tile.TileContext(nc) as tc:
        with (
            tc.tile_pool(name="persist", bufs=1) as pp,
            tc.tile_pool(name="ps_s", bufs=2, space="PSUM") as ps_s,
            tc.tile_pool(name="ps_pv", bufs=2, space="PSUM") as ps_pv,
            tc.tile_pool(name="ps_misc", bufs=2, space="PSUM") as ps_misc,
            tc.tile_pool(name="psb", bufs=4) as pexp,
            tc.tile_pool(name="small", bufs=2) as psm,
            tc.tile_pool(name="outp", bufs=3) as pout,
        ):
            # ---- persistent sbuf tensors -------------------------------
            # xT[p, cc, t] = x[t, cc*128 + p] (host supplies x^T in bf16)
            xT = pp.tile([128, CC, T], BF16, tag="xT")
            qT = pp.tile([128, 2, T], BF16, tag="qT")   # [p, half, t]
            kT = pp.tile([128, 2, T], BF16, tag="kT")
            vA = pp.tile([128, TC, HPC * (DH + 1)], BF16, tag="vA")
            yT = pp.tile([128, 2, T], BF16, tag="yT")
            wo_sb = pp.tile([128, 2, C], BF16, tag="wo")
            wq_sb = pp.tile([128, CC, DQC], BF16, tag="wq")
            wk_sb = pp.tile([128, CC, DQC], BF16, tag="wk")
            wv_sb = pp.tile([128, CC, DQC], BF16, tag="wv")
            wqf = pp.tile([128, CC, DQC], F32, tag="wqf")
            wkf = pp.tile([128, CC, DQC], F32, tag="wkf")
            wvf = pp.tile([128, CC, DQC], F32, tag="wvf")
            wof = pp.tile([128, 2, C], F32, tag="wof")
            bqs = pp.tile([128, 2], F32, tag="bqs")
            bks = pp.tile([128, 2], F32, tag="bks")
            bvs = pp.tile([128, DQC], F32, tag="bvs")

            # x^T first: plain contiguous DMAs per 512-t window
            # (everything downstream depends on these)
            xt16_v = xt16.ap().rearrange("(cc p) t -> p cc t", p=128)
            for w in range(TJ):
                nc.sync.dma_start(
                    xT[:, :, ts(w, 512)], xt16_v[:, :, ts(w, 512)]
                )

            # tri01[p, f] = 1 where f >= p else 0 (keep s<=t in diag blk)
            tri01 = pp.tile([128, 128], BF16, tag="tri01")
            nc.gpsimd.memset(tri01[:], 1.0)
            nc.gpsimd.affine_select(
                out=tri01[:],
                in_=tri01[:],
                compare_op=ALU.is_ge,
                fill=0.0,
                base=0,
                # keep where (-1)*p + f >= 0, i.e. f >= p
                pattern=[[1, 128]],
                channel_multiplier=-1,
            )

            # ones column of v_aug (softmax denominator via PV matmul)
            vA4 = vA[:].rearrange("p s (h d) -> p s h d", d=DH + 1)
            onesf = pp.tile([128, TC * HPC], F32, tag="onesf")
            nc.gpsimd.memset(onesf[:], 1.0)
            nc.vector.tensor_copy(
                vA4[:, :, :, DH : DH + 1],
                onesf[:].rearrange("p (s h o) -> p s h o", h=HPC, o=1),
            )

            # biases + weights on the scalar-engine DMA queue (parallel
            # with the x transposes on the sync queue)
            nc.scalar.dma_start(bqs[:], bq.ap().rearrange("(k p) -> p k", p=128))
            nc.scalar.dma_start(bks[:], bk.ap().rearrange("(k p) -> p k", p=128))
            nc.scalar.dma_start(
                bvs[0:1, :], bv.ap().rearrange("(o n) -> o n", o=1)
            )
            nc.gpsimd.partition_broadcast(bvs[:], bvs[0:1, :])

            nc.scalar.dma_start(
                wqf[:], wq.ap().rearrange("(c p) d -> p c d", p=128)
            )
            nc.scalar.dma_start(
                wkf[:], wk.ap().rearrange("(c p) d -> p c d", p=128)
            )
            nc.scalar.dma_start(
                wvf[:], wv.ap().rearrange("(c p) d -> p c d", p=128)
            )
            nc.scalar.dma_start(
                wof[:], wo.ap().rearrange("(k p) n -> p k n", p=128)
            )
            nc.vector.tensor_copy(wq_sb[:], wqf[:])
            nc.vector.tensor_copy(wk_sb[:], wkf[:])
            nc.vector.tensor_copy(wv_sb[:], wvf[:])
            nc.vector.tensor_copy(wo_sb[:], wof[:])

            # ---- filler generators (PE work to hide exp latency) -------
            def gen_qk_proj(w, hp, wsb, bias, dst):
                pq = ps_misc.tile([128, 512], F32, tag="misc", name=f"pj{w}{hp}")
                for cc in range(CC):
                    nc.tensor.matmul(
                        pq[:],
                        wsb[:, cc, ts(hp, 128)],
                        xT[:, cc, ts(w, 512)],
                        start=(cc == 0),
                        stop=(cc == CC - 1),
                    )
                    yield
                nc.vector.tensor_scalar(
                    dst[:, hp, ts(w, 512)],
                    pq[:],
                    bias[:, hp : hp + 1],
                    None,
                    ALU.add,
                )

            def gen_v_proj(sc):
                pv = ps_misc.tile([128, 512], F32, tag="misc", name=f"pv{sc}")
                for cc in range(CC):
                    nc.tensor.matmul(
                        pv[:, :DQC],
                        xT[:, cc, ts(sc, 128)],
                        wv_sb[:, cc, :],
                        start=(cc == 0),
                        stop=(cc == CC - 1),
                    )
                    yield
                nc.vector.tensor_tensor(
                    vA4[:, sc, :, :DH],
                    pv[:, :DQC].rearrange("p (h d) -> p h d", d=DH),
                    bvs[:].rearrange("p (h d) -> p h d", d=DH),
                    ALU.add,
                )

            def gen_o_proj(w, tt):
                t0 = 512 * w + 128 * tt
                ot = pout.tile([128, C], F32, tag="o", name=f"o{w}{tt}")
                for nb in range(2):
                    po = ps_misc.tile(
                        [128, 512], F32, tag="misc", name=f"po{w}{tt}{nb}"
                    )
                    for kk in range(2):
                        nc.tensor.matmul(
                            po[:],
                            yT[:, kk, ds(t0, 128)],
                            wo_sb[:, kk, ts(nb, 512)],
                            start=(kk == 0),
                            stop=(kk == 1),
                        )
                        yield
                    nc.vector.tensor_copy(ot[:, ts(nb, 512)], po[:])
                    yield
                nc.sync.dma_start(out.ap()[ds(t0, 128), :], ot[:])

            def drain(q, n):
                steps = 0
                while q and steps < n:
                    try:
                        next(q[0])
                    except StopIteration:
                        q.pop(0)
                        continue
                    steps += 1

            # projections for window 0 (emitted directly)
            for g in (
                [gen_qk_proj(0, hp, wq_sb, bqs, qT) for hp in range(2)]
                + [gen_qk_proj(0, hp, wk_sb, bks, kT) for hp in range(2)]
                + [gen_v_proj(sc) for sc in range(4)]
            ):
                for _ in g:
                    pass

            # ---- window loop: attend w, filling with w+1 proj etc ------
            for w in range(TJ):
                fillers = []
                if w + 1 < TJ:
                    for hp in range(2):
                        fillers.append(gen_qk_proj(w + 1, hp, wq_sb, bqs, qT))
                        fillers.append(gen_qk_proj(w + 1, hp, wk_sb, bks, kT))
                    for sc in range(4 * (w + 1), 4 * (w + 1) + 4):
                        fillers.append(gen_v_proj(sc))
                # o-projections all deferred to att(3), the filler-starved
                # window (att(0..2) are fed by next-window projections)
                o_wins = {3: [0, 1, 2]}.get(w, [])
                for ow in o_wins:
                    for tt in range(4):
                        fillers.append(gen_o_proj(ow, tt))

                n_sc = 4 * (w + 1)
                total_steps = (64 if w + 1 < TJ else 0) + 24 * len(o_wins)
                # hold back ~24 steps for the two segment tails, where the
                # scalar engine's exp lag would otherwise stall the PE
                per_iter = max(1, (total_steps - 24) // (2 * n_sc))

                for hp in range(2):
                    hA, hB = 2 * hp, 2 * hp + 1
                    ppv_A = ps_pv.tile([128, 512], F32, tag="pv", name=f"pA{w}{hp}")
                    ppv_B = ps_pv.tile([128, 512], F32, tag="pv", name=f"pB{w}{hp}")

                    def emit_pv(sc, psb, off):
                        for hi, h in ((0, hA), (1, hB)):
                            ppv = ppv_A if hi == 0 else ppv_B
                            nc.tensor.matmul(
                                ppv[: DH + 1, ds(off, 512 - off)],
                                vA[:, sc, ds(h * (DH + 1), DH + 1)],
                                psb[:, ds(512 * hi + off, 512 - off)],
                                start=(sc == 0),
                                stop=(sc == n_sc - 1),
                            )

                    pending = []
                    for sc in range(n_sc):
                        k = sc - 4 * w  # >=0 on the causal diagonal
                        off = 128 * k if k > 0 else 0
                        pss = ps_s.tile([128, 1024], F32, tag="s", name=f"s{w}{hp}{sc}")
                        for hi in range(2):
                            half = 512 * hi
                            prow = slice(64 * hi, 64 * hi + 64)
                            nc.tensor.matmul(
                                pss[:, ds(half + off, 512 - off)],
                                kT[prow, hp, ts(sc, 128)],
                                qT[prow, hp, ds(512 * w + off, 512 - off)],
                                start=True,
                                stop=True,
                                tile_position=(64 * hi, 0),
                            )
                        psb = pexp.tile([128, 1024], BF16, tag="p", name=f"e{w}{hp}{sc}")
                        if off == 0:
                            nc.scalar.activation(psb[:], pss[:], AF.Exp)
                        else:
                            nc.scalar.activation(
                                psb[:, ds(off, 512 - off)],
                                pss[:, ds(off, 512 - off)],
                                AF.Exp,
                            )
                            nc.scalar.activation(
                                psb[:, ds(512 + off, 512 - off)],
                                pss[:, ds(512 + off, 512 - off)],
                                AF.Exp,
                            )
                        if k >= 0:
                            # zero s>t inside the diagonal blocks
                            for hi in range(2):
                                blk = ds(512 * hi + off, 128)
                                nc.gpsimd.tensor_tensor(
                                    psb[:, blk], psb[:, blk], tri01[:], ALU.mult
                                )
                        pending.append((sc, psb, off))
                        if len(pending) > 2:
                            emit_pv(*pending.pop(0))
                        # reserve the last iterations' quota for the tail
                        if sc < n_sc - 2:
                            drain(fillers, per_iter)
                    for args in pending:
                        emit_pv(*args)
                        drain(fillers, 6)

                    # normalize: y^T = Y_unnorm^T * (1/denom)
                    den = psm.tile([1, 1024], F32, tag="den")
                    for hi, ppv in ((0, ppv_A), (1, ppv_B)):
                        nc.vector.tensor_copy(
                            den[:, ts(hi, 512)], ppv[DH : DH + 1, :]
                        )
                    rec = psm.tile([1, 1024], F32, tag="rec")
                    scr = psm.tile([1, 1024], F32, tag="scr")
                    nc.vector.reciprocal_approx_accurate(rec[:], den[:], scr[:])
                    recB = psm.tile([DH, 1024], F32, tag="recB")
                    nc.gpsimd.partition_broadcast(recB[:], rec[:])
                    for hi, h in ((0, hA), (1, hB)):
                        ppv = ppv_A if hi == 0 else ppv_B
                        nc.vector.tensor_tensor(
                            yT[ds(64 * (h % 2), DH), h // 2, ts(w, 512)],
                            ppv[:DH, :],
                            recB[:, ts(hi, 512)],
                            ALU.mult,
                        )
                    drain(fillers, 12)
                drain(fillers, 1 << 30)

            # last window's o-projection
            for tt in range(4):
                for _ in gen_o_proj(TJ - 1, tt):
                    pass

    nc.compile()
    return nc


_CACHE = {}


def _get_program():
    if "nc" not in _CACHE:
        _CACHE["nc"] = build_program()
    return _CACHE["nc"]


def make_in_maps(x, wq, bq, wk, bk, wv, bv, wo):
    import ml_dtypes

    scale = np.float32(1.0 / np.sqrt(DH))
    xt16 = [
        np.ascontiguousarray(x[b].T).astype(ml_dtypes.bfloat16)
        for b in range(B)
    ]
    in_maps = []
    for core in range(N_CORES):
        b, g = core // 4, core % 4
        sl = slice(g * DQC, (g + 1) * DQC)
        in_maps.append(
            {
                "xt16": xt16[b],
                "wq": np.ascontiguousarray(wq[:, sl]) * scale,
                "wk": np.ascontiguousarray(wk[:, sl]),
                "wv": np.ascontiguousarray(wv[:, sl]),
                "wo": np.ascontiguousarray(wo[sl, :]),
                "bq": np.ascontiguousarray(bq[sl]) * scale,
                "bk": np.ascontiguousarray(bk[sl]),
                "bv": np.ascontiguousarray(bv[sl]),
            }
        )
    return in_maps


def kernel(x, wq, bq, wk, bk, wv, bv, wo, bo):
    from concourse import bass_utils

    x = np.asarray(x, dtype=np.float32)
    wq = np.asarray(wq, dtype=np.float32)
    wk = np.asarray(wk, dtype=np.float32)
    wv = np.asarray(wv, dtype=np.float32)
    wo = np.asarray(wo, dtype=np.float32)
    bq = np.asarray(bq, dtype=np.float32)
    bk = np.asarray(bk, dtype=np.float32)
    bv = np.asarray(bv, dtype=np.float32)
    bo = np.asarray(bo, dtype=np.float32)

    nc = _get_program()
    in_maps = make_in_maps(x, wq, bq, wk, bk, wv, bv, wo)
    res = bass_utils.run_bass_kernel_spmd(
        nc, in_maps, core_ids=list(range(N_CORES))
    )
    y = np.zeros((B, T, C), dtype=np.float32)
    for core in range(N_CORES):
        y[core // 4] += res.results[core]["out"]
    y += bo
    return y
